# revision 14
# baseline (speedup 1.0000x reference)
"""HGNN conv on 8 trn2 cores — v2 (count-bucketed batched gathers).

out = D_v^-1 H D_e^-1 H^T input W + bias   (W applied to edge features y)

Phase A (edge-sharded): edges sorted by incidence count (desc) and dealt
round-robin to 8 cores in 128-edge windows; window w holds 128 edges of
near-equal count, padded to K_A[w] slots. ONE indirect DMA gathers all
K_A[w]*128 input rows for the window ([P, K*D] tile); a pairwise bf16
add-tree reduces over K; recip-scale on the Act engine; W applied via
PE transpose + matmul; result rows staged to y_shard. Every 5 windows a
chunk AllGathers straight into the Shared y_full tensor.
Phase B (node-sharded): same bucketed scheme over nodes; gather y rows
per entry from y_full, tree-reduce, recip-scale, add bias, store.

Padding gathers read a host-appended zero row (input) / a zero y row.
"""
import os
import sys

for _p in ('/opt/trn_rl_repo', '/root/.axon_site/_ro/trn_rl_repo'):
    if os.path.isdir(_p) and _p not in sys.path:
        sys.path.insert(0, _p)

import numpy as np

P = 128
NCORES = 8
N_NODE = 50000
N_EDGE = 25000
D = 128
W_A = 25              # edge windows per core (8*25*128 = 25600 slots)
RANKS_E = NCORES * W_A * P
ESH = W_A * P         # 3200 edge slots per core
NCH = 5               # allgather chunks (5 windows each)
CH_E = ESH // NCH     # 640 edge rows per chunk per core
W_B = 49              # node windows per core (8*49*128 = 50176 slots)
RANKS_V = NCORES * W_B * P
NSH = W_B * P         # 6272 node slots per core

_PROG_CACHE = {}
LAST_RESULTS = None


def _y_row(rank):
    """y_full row for global edge rank, chunk-major allgather layout."""
    c = (rank // P) % NCORES
    w = rank // (NCORES * P)
    p = rank % P
    k = w // (W_A // NCH)
    return (k * (NCORES * CH_E) + c * CH_E + (w % (W_A // NCH)) * P + p)


def _preprocess(V, E):
    V = np.asarray(V).astype(np.int64)
    E = np.asarray(E).astype(np.int64)
    nnz = len(V)

    cntE = np.bincount(E, minlength=N_EDGE)
    cntV = np.bincount(V, minlength=N_NODE)

    # ----- Phase A: bucket edges by count (desc) -----
    eord = np.argsort(-cntE, kind='stable')          # rank -> edge id
    rankE = np.empty(N_EDGE, np.int64)
    rankE[eord] = np.arange(N_EDGE)                  # edge id -> rank
    cnt_r = np.zeros(RANKS_E, np.int64)
    cnt_r[:N_EDGE] = cntE[eord]
    blk = NCORES * P                                 # 1024 ranks per window row
    K_A = [max(1, int(cnt_r[w * blk])) for w in range(W_A)]
    baseA = np.concatenate([[0], np.cumsum(K_A)]).astype(np.int64)
    SA = int(baseA[-1])

    offsA = np.full((NCORES, P, SA), N_NODE, np.int32)   # pad -> zero row
    ordA = np.argsort(rankE[E], kind='stable')
    rs = rankE[E[ordA]]
    starts = np.searchsorted(rs, np.arange(RANKS_E))
    j = np.arange(nnz) - starts[rs]
    c = (rs // P) % NCORES
    w = rs // blk
    p = rs % P
    offsA[c, p, baseA[w] + j] = V[ordA]

    recipA_flat = np.where(cnt_r > 0, 1.0 / np.maximum(cnt_r, 1), 0.0)
    recipA = recipA_flat.reshape(W_A, NCORES, P).transpose(1, 2, 0)
    recipA = np.ascontiguousarray(recipA, np.float32)

    # ----- Phase B: bucket nodes by count (desc) -----
    nord = np.argsort(-cntV, kind='stable')          # rank -> node id
    rankV = np.empty(N_NODE, np.int64)
    rankV[nord] = np.arange(N_NODE)
    cnt2_r = np.zeros(RANKS_V, np.int64)
    cnt2_r[:N_NODE] = cntV[nord]
    K_B = [max(1, int(cnt2_r[w * blk])) for w in range(W_B)]
    baseB = np.concatenate([[0], np.cumsum(K_B)]).astype(np.int64)
    SB = int(baseB[-1])

    pad_row = _y_row(RANKS_E - 1)                    # a guaranteed-zero y row
    offsB = np.full((NCORES, P, SB), pad_row, np.int32)
    ordB = np.argsort(rankV[V], kind='stable')
    rs2 = rankV[V[ordB]]
    starts2 = np.searchsorted(rs2, np.arange(RANKS_V))
    j2 = np.arange(nnz) - starts2[rs2]
    c2 = (rs2 // P) % NCORES
    w2 = rs2 // blk
    p2 = rs2 % P
    offsB[c2, p2, baseB[w2] + j2] = _y_row(rankE[E[ordB]])

    recipB_flat = np.where(cnt2_r > 0, 1.0 / np.maximum(cnt2_r, 1), 0.0)
    recipB = recipB_flat.reshape(W_B, NCORES, P).transpose(1, 2, 0)
    recipB = np.ascontiguousarray(recipB, np.float32)

    return dict(K_A=K_A, K_B=K_B, baseA=baseA, baseB=baseB,
                offsA=offsA, recipA=recipA, offsB=offsB, recipB=recipB,
                nord=nord)


def _emulate(pp, inp_f32, weight, bias):
    """Numpy emulation of the exact device program (for logic validation)."""
    K_A, K_B = pp['K_A'], pp['K_B']
    baseA, baseB = pp['baseA'], pp['baseB']
    inpz = np.concatenate([inp_f32, np.zeros((1, D), np.float32)], 0)
    y_full = np.zeros((RANKS_E, D), np.float32)
    for c in range(NCORES):
        for w in range(W_A):
            offs = pp['offsA'][c][:, baseA[w]:baseA[w] + K_A[w]]
            g = inpz[offs]                       # [P, K, D]
            s = g.sum(1)
            y = s * pp['recipA'][c][:, w][:, None]
            yw = y @ weight
            for p in range(P):
                rank = w * NCORES * P + c * P + p
                y_full[_y_row(rank)] = yw[p]
    out = np.zeros((N_NODE, D), np.float32)
    for c in range(NCORES):
        for w in range(W_B):
            offs = pp['offsB'][c][:, baseB[w]:baseB[w] + K_B[w]]
            g = y_full[offs]
            s = g.sum(1)
            z = s * pp['recipB'][c][:, w][:, None] + bias[None, :]
            for p in range(P):
                rank = w * NCORES * P + c * P + p
                if rank < N_NODE:
                    out[pp['nord'][rank]] = z[p]
    return out


def _build_program(K_A, K_B, rep=1, no_coll=False, dbg=False):
    import concourse.bacc as bacc
    import concourse.bass as bass
    import concourse.tile as tile
    from concourse import mybir

    f32 = mybir.dt.float32
    bf16 = mybir.dt.bfloat16
    i32 = mybir.dt.int32
    add = mybir.AluOpType.add
    Copy = mybir.ActivationFunctionType.Copy

    SA, SB = sum(K_A), sum(K_B)
    baseA = np.concatenate([[0], np.cumsum(K_A)]).astype(int)
    baseB = np.concatenate([[0], np.cumsum(K_B)]).astype(int)

    nc = bacc.Bacc(None, target_bir_lowering=False, debug=False)
    inpz = nc.dram_tensor("inpz", [N_NODE + 1, D], bf16, kind="ExternalInput")
    wgt = nc.dram_tensor("wgt", [D, D], bf16, kind="ExternalInput")
    bias_bc = nc.dram_tensor("bias_bc", [P, D], f32, kind="ExternalInput")
    ident_in = nc.dram_tensor("ident_in", [P, P], bf16, kind="ExternalInput")
    offsA_d = nc.dram_tensor("offsA", [P, SA], i32, kind="ExternalInput")
    recipA_d = nc.dram_tensor("recipA", [P, W_A], f32, kind="ExternalInput")
    offsB_d = nc.dram_tensor("offsB", [P, SB], i32, kind="ExternalInput")
    recipB_d = nc.dram_tensor("recipB", [P, W_B], f32, kind="ExternalInput")
    out = nc.dram_tensor("out", [NSH, D], f32, kind="ExternalOutput")
    if dbg:
        dbg_g = nc.dram_tensor("dbg_g", [P, K_A[0] * D], bf16,
                               kind="ExternalOutput")
        dbg_s = nc.dram_tensor("dbg_s", [P, D], f32, kind="ExternalOutput")
        dbg_ys = nc.dram_tensor("dbg_ys", [ESH, D], bf16,
                                kind="ExternalOutput")
        dbg_yf = nc.dram_tensor("dbg_yf", [RANKS_E, D], bf16,
                                kind="ExternalOutput")

    with tile.TileContext(nc) as tc:
        with tc.tile_pool(name="const", bufs=1) as cpool, \
             tc.tile_pool(name="gat", bufs=3) as gpool, \
             tc.tile_pool(name="sred", bufs=3) as spool, \
             tc.tile_pool(name="yst", bufs=4) as ypool, \
             tc.tile_pool(name="res", bufs=4) as respool, \
             tc.tile_pool(name="ps", bufs=4, space="PSUM") as ppool, \
             tc.tile_pool(name="dram", bufs=1, space="DRAM") as dpool:

            wgt_t = cpool.tile([D, D], bf16)
            nc.sync.dma_start(out=wgt_t[:], in_=wgt[:])
            bias_t = cpool.tile([P, D], f32)
            nc.sync.dma_start(out=bias_t[:], in_=bias_bc[:])
            ident_t = cpool.tile([P, P], bf16)
            nc.sync.dma_start(out=ident_t[:], in_=ident_in[:])
            offsA_t = cpool.tile([P, SA], i32)
            nc.sync.dma_start(out=offsA_t[:], in_=offsA_d[:])
            recipA_t = cpool.tile([P, W_A], f32)
            nc.sync.dma_start(out=recipA_t[:], in_=recipA_d[:])
            offsB_t = cpool.tile([P, SB], i32)
            nc.sync.dma_start(out=offsB_t[:], in_=offsB_d[:])
            recipB_t = cpool.tile([P, W_B], f32)
            nc.sync.dma_start(out=recipB_t[:], in_=recipB_d[:])

            y_shard = dpool.tile([ESH, D], bf16)
            y_full = dpool.tile([RANKS_E, D], bf16, name="y_full")
            y_ch_all = [[dpool.tile([NCORES * CH_E, D], bf16,
                                    addr_space="Shared",
                                    name=f"y_ch{k}_{r}") for k in range(NCH)]
                        for r in range(rep)]

            def tree_reduce(g, K, tag):
                """Pairwise bf16 add-tree over K blocks of width D; returns
                an AP [P, D] (bf16 if K==1 else f32 tile)."""
                blocks = K
                while blocks > 2:
                    h = blocks // 2
                    nc.vector.tensor_tensor(
                        out=g[:, :h * D], in0=g[:, :h * D],
                        in1=g[:, (blocks - h) * D:blocks * D], op=add)
                    blocks -= h
                if blocks == 2:
                    s = spool.tile([P, D], f32, name=f"s_{tag}", tag=f"s_{tag}")
                    nc.vector.tensor_tensor(out=s[:], in0=g[:, :D],
                                            in1=g[:, D:2 * D], op=add)
                    return s[:]
                return g[:, :D]

            for _r in range(rep):
                y_ch = y_ch_all[_r]
                # ---------------- Phase A ----------------
                for w in range(W_A):
                    KA = K_A[w]
                    g = gpool.tile([P, KA * D], bf16, name="gA", tag="gA")
                    for k in range(KA):
                        nc.gpsimd.indirect_dma_start(
                            out=g[:, k * D:(k + 1) * D],
                            out_offset=None, in_=inpz[:],
                            in_offset=bass.IndirectOffsetOnAxis(
                                ap=offsA_t[:, baseA[w] + k:baseA[w] + k + 1],
                                axis=0))
                    if dbg and w == 0 and _r == 0:
                        nc.sync.dma_start(out=dbg_g[:], in_=g[:])
                    src = tree_reduce(g, KA, "a")
                    if dbg and w == 0 and _r == 0:
                        nc.sync.dma_start(out=dbg_s[:], in_=src)
                    y = ypool.tile([P, D], bf16, name="yA", tag="yA")
                    nc.scalar.activation(out=y[:], in_=src, func=Copy,
                                         scale=recipA_t[:, w:w + 1])
                    yT_p = ppool.tile([P, D], bf16, name="yTp", tag="yTp")
                    nc.tensor.transpose(out=yT_p[:], in_=y[:],
                                        identity=ident_t[:])
                    yT = ypool.tile([P, D], bf16, name="yT", tag="yT")
                    nc.scalar.activation(out=yT[:], in_=yT_p[:], func=Copy)
                    yw_p = ppool.tile([P, D], f32, name="ywp", tag="ywp")
                    nc.tensor.matmul(yw_p[:], lhsT=yT[:], rhs=wgt_t[:],
                                     start=True, stop=True)
                    yw = ypool.tile([P, D], bf16, name="ywA", tag="ywA")
                    nc.scalar.activation(out=yw[:], in_=yw_p[:], func=Copy)
                    nc.sync.dma_start(out=y_shard[w * P:(w + 1) * P, :],
                                      in_=yw[:])
                    if (w + 1) % (W_A // NCH) == 0:
                        k = w // (W_A // NCH)
                        if not no_coll:
                            nc.gpsimd.collective_compute(
                                "AllGather", mybir.AluOpType.bypass,
                                replica_groups=[list(range(NCORES))],
                                ins=[y_shard[k * CH_E:(k + 1) * CH_E, :]],
                                outs=[y_ch[k].opt()])
                        nc.sync.dma_start(
                            out=y_full[k * NCORES * CH_E:
                                       (k + 1) * NCORES * CH_E, :],
                            in_=y_ch[k][:])

                # ---------------- Phase B ----------------
                for w in range(W_B):
                    KB = K_B[w]
                    g2 = gpool.tile([P, KB * D], bf16, name="gB", tag="gB")
                    for k in range(KB):
                        nc.gpsimd.indirect_dma_start(
                            out=g2[:, k * D:(k + 1) * D],
                            out_offset=None, in_=y_full[:],
                            in_offset=bass.IndirectOffsetOnAxis(
                                ap=offsB_t[:, baseB[w] + k:baseB[w] + k + 1],
                                axis=0))
                    src2 = tree_reduce(g2, KB, "b")
                    z = respool.tile([P, D], f32, name="zB", tag="zB")
                    nc.scalar.activation(out=z[:], in_=src2, func=Copy,
                                         scale=recipB_t[:, w:w + 1])
                    res = respool.tile([P, D], f32, name="resB", tag="resB")
                    nc.vector.tensor_tensor(out=res[:], in0=z[:],
                                            in1=bias_t[:], op=add)
                    nc.sync.dma_start(out=out[w * P:(w + 1) * P, :],
                                      in_=res[:])

            if dbg:
                nc.sync.dma_start(out=dbg_ys[:], in_=y_shard[:])
                nc.sync.dma_start(out=dbg_yf[:], in_=y_full[:])

    nc.compile()
    return nc


def kernel(input, weight, bias, V, E, num_edges):
    global LAST_RESULTS
    inp = np.ascontiguousarray(np.asarray(input), dtype=np.float32)
    wgt = np.ascontiguousarray(np.asarray(weight), dtype=np.float32)
    b = np.asarray(bias).astype(np.float32)
    pp = _preprocess(V, E)

    if os.environ.get('KERNEL_EMULATE'):
        return _emulate(pp, inp, wgt, b)

    from concourse.bass_utils import run_bass_kernel_spmd

    key = (tuple(pp['K_A']), tuple(pp['K_B']))
    if key not in _PROG_CACHE:
        _PROG_CACHE[key] = _build_program(*key)
    nc = _PROG_CACHE[key]

    import ml_dtypes
    bf = ml_dtypes.bfloat16
    inpz = np.concatenate([inp, np.zeros((1, D), np.float32)],
                          0).astype(bf)
    bias_bc = np.tile(b[None, :], (P, 1)).astype(np.float32)
    ident = np.eye(P, dtype=np.float32).astype(bf)
    in_maps = []
    for c in range(NCORES):
        in_maps.append(dict(
            inpz=inpz, wgt=wgt.astype(bf), bias_bc=bias_bc, ident_in=ident,
            offsA=pp['offsA'][c], recipA=pp['recipA'][c],
            offsB=pp['offsB'][c], recipB=pp['recipB'][c]))

    trace = bool(os.environ.get('KERNEL_TRACE'))
    res = run_bass_kernel_spmd(nc, in_maps, list(range(NCORES)), trace=trace)
    LAST_RESULTS = res

    out_full = np.zeros((N_NODE, D), np.float32)
    nord = pp['nord']
    for c in range(NCORES):
        rows = np.asarray(res.results[c]['out'])          # [NSH, D]
        ranks = (np.arange(NSH) // P) * (NCORES * P) + c * P + np.arange(NSH) % P
        m = ranks < N_NODE
        out_full[nord[ranks[m]]] = rows[m]
    return out_full


# revision 17
# speedup vs baseline: 1.2285x; 1.2285x over previous
"""HGNN conv on 8 trn2 cores — v3 (dma_gather batched window gathers).

out = D_v^-1 H D_e^-1 H^T input W + bias   (W applied to edge features y)

Phase A (edge-sharded): edges sorted by (lo,hi) incidence counts (desc,
where lo counts entries with V < 32767 and hi the rest — dma_gather
indices are int16 so the 50001-row input is split into inp_lo/inp_hi,
each with a trailing zero row for padding) and dealt round-robin to 8
cores in 128-edge windows. Per window: one dma_gather from inp_lo
(K_lo*128 rows) + one from inp_hi (K_hi*128 rows) into a [P, K*D] tile;
pairwise bf16 add-tree reduces over K; recip-scale on the Act engine; W
applied via PE transpose + matmul; rows staged to y_shard. Every 5
windows a chunk AllGathers into Shared y_ch -> copied into y_full.
Phase B (node-sharded): nodes bucketed by count; one dma_gather per
window from y_full (25600 rows, int16-safe), tree-reduce, recip-scale,
add bias, store. Output rows are host-side inverse-permuted.
"""
import os
import sys

for _p in ('/opt/trn_rl_repo', '/root/.axon_site/_ro/trn_rl_repo'):
    if os.path.isdir(_p) and _p not in sys.path:
        sys.path.insert(0, _p)

import numpy as np

P = 128
NCORES = 8
N_NODE = 50000
N_EDGE = 25000
D = 128
SPLIT = 32767         # input rows [0, SPLIT) -> inp_lo, [SPLIT, N) -> inp_hi
LO_ROWS = SPLIT + 1   # + zero row at index SPLIT
HI_ROWS = N_NODE - SPLIT + 1  # + zero row at index N_NODE - SPLIT
W_A = 25              # edge windows per core (8*25*128 = 25600 slots)
RANKS_E = NCORES * W_A * P
ESH = W_A * P         # 3200 edge slots per core
NCH = 5               # allgather chunks (5 windows each)
CH_E = ESH // NCH     # 640 edge rows per chunk per core
W_B = 49              # node windows per core (8*49*128 = 50176 slots)
RANKS_V = NCORES * W_B * P
NSH = W_B * P         # 6272 node slots per core

_PROG_CACHE = {}
LAST_RESULTS = None


def _y_row(rank):
    """y_full row for global edge rank, chunk-major allgather layout."""
    c = (rank // P) % NCORES
    w = rank // (NCORES * P)
    p = rank % P
    k = w // (W_A // NCH)
    return (k * (NCORES * CH_E) + c * CH_E + (w % (W_A // NCH)) * P + p)


def _pack_idx(mat):
    """[P, K] int -> [128, 8*K] int16: flat index i=k*128+p lives at
    [i%16, i//16], replicated across the 8 16-partition groups."""
    Pn, K = mat.shape
    assert Pn == P
    flat = mat.T.reshape(-1)             # i = k*128 + p
    arr = flat.reshape(-1, 16).T         # [16, 8*K]
    return np.ascontiguousarray(np.tile(arr, (8, 1)).astype(np.int16))


def _fill_cols(dst, base, K, rs, j, vals, blk):
    """Scatter vals into dst[c][p, 8-packed cols] — done at int-matrix level
    by the caller; helper kept trivial."""
    raise NotImplementedError


def _preprocess(V, E):
    V = np.asarray(V).astype(np.int64)
    E = np.asarray(E).astype(np.int64)
    nnz = len(V)

    cntV = np.bincount(V, minlength=N_NODE)
    is_lo = V < SPLIT
    lo_cnt = np.bincount(E[is_lo], minlength=N_EDGE)
    hi_cnt = np.bincount(E[~is_lo], minlength=N_EDGE)

    # ----- Phase A: bucket edges by (lo, hi) counts desc -----
    eord = np.lexsort((-hi_cnt, -lo_cnt))            # rank -> edge id
    rankE = np.empty(N_EDGE, np.int64)
    rankE[eord] = np.arange(N_EDGE)
    lo_r = np.zeros(RANKS_E, np.int64)
    lo_r[:N_EDGE] = lo_cnt[eord]
    hi_r = np.zeros(RANKS_E, np.int64)
    hi_r[:N_EDGE] = hi_cnt[eord]
    blk = NCORES * P                                 # 1024 ranks per window row
    K_lo, K_hi = [], []
    for w in range(W_A):
        kl = int(lo_r[w * blk:(w + 1) * blk].max())
        kh = int(hi_r[w * blk:(w + 1) * blk].max())
        if kl + kh == 0:
            kh = 1
        K_lo.append(kl)
        K_hi.append(kh)
    baseLo = np.concatenate([[0], np.cumsum(K_lo)]).astype(np.int64)
    baseHi = np.concatenate([[0], np.cumsum(K_hi)]).astype(np.int64)
    SLo, SHi = int(baseLo[-1]), int(baseHi[-1])

    offsLo = np.full((NCORES, P, SLo), SPLIT, np.int64)
    offsHi = np.full((NCORES, P, SHi), N_NODE - SPLIT, np.int64)

    def scatter(mask, offs, base, vals):
        ordx = np.argsort(rankE[E[mask]], kind='stable')
        rsx = rankE[E[mask]][ordx]
        startsx = np.searchsorted(rsx, np.arange(RANKS_E))
        jx = np.arange(len(rsx)) - startsx[rsx]
        cx = (rsx // P) % NCORES
        wx = rsx // blk
        px = rsx % P
        offs[cx, px, base[wx] + jx] = vals[mask][ordx]

    scatter(is_lo, offsLo, baseLo, V)
    scatter(~is_lo, offsHi, baseHi, V - SPLIT)

    cnt_r = lo_r + hi_r
    recipA_flat = np.where(cnt_r > 0, 1.0 / np.maximum(cnt_r, 1), 0.0)
    recipA = recipA_flat.reshape(W_A, NCORES, P).transpose(1, 2, 0)
    recipA = np.ascontiguousarray(recipA, np.float32)

    # ----- Phase B: bucket nodes by count desc -----
    nord = np.argsort(-cntV, kind='stable')
    rankV = np.empty(N_NODE, np.int64)
    rankV[nord] = np.arange(N_NODE)
    cnt2_r = np.zeros(RANKS_V, np.int64)
    cnt2_r[:N_NODE] = cntV[nord]
    K_B = [max(1, int(cnt2_r[w * blk])) for w in range(W_B)]
    baseB = np.concatenate([[0], np.cumsum(K_B)]).astype(np.int64)
    SB = int(baseB[-1])

    pad_row = _y_row(RANKS_E - 1)                    # a guaranteed-zero y row
    offsB = np.full((NCORES, P, SB), pad_row, np.int64)
    ordB = np.argsort(rankV[V], kind='stable')
    rs2 = rankV[V[ordB]]
    starts2 = np.searchsorted(rs2, np.arange(RANKS_V))
    j2 = np.arange(nnz) - starts2[rs2]
    c2 = (rs2 // P) % NCORES
    w2 = rs2 // blk
    p2 = rs2 % P
    offsB[c2, p2, baseB[w2] + j2] = _y_row(rankE[E[ordB]])

    recipB_flat = np.where(cnt2_r > 0, 1.0 / np.maximum(cnt2_r, 1), 0.0)
    recipB = recipB_flat.reshape(W_B, NCORES, P).transpose(1, 2, 0)
    recipB = np.ascontiguousarray(recipB, np.float32)

    # pack per-core int16 index tiles
    idxLo = np.stack([np.concatenate(
        [_pack_idx(offsLo[c][:, baseLo[w]:baseLo[w + 1]]) for w in range(W_A)
         if K_lo[w] > 0] or [np.zeros((P, 0), np.int16)], 1)
        for c in range(NCORES)])
    idxHi = np.stack([np.concatenate(
        [_pack_idx(offsHi[c][:, baseHi[w]:baseHi[w + 1]]) for w in range(W_A)
         if K_hi[w] > 0] or [np.zeros((P, 0), np.int16)], 1)
        for c in range(NCORES)])
    idxB = np.stack([np.concatenate(
        [_pack_idx(offsB[c][:, baseB[w]:baseB[w + 1]]) for w in range(W_B)], 1)
        for c in range(NCORES)])

    return dict(K_lo=K_lo, K_hi=K_hi, K_B=K_B,
                baseLo=baseLo, baseHi=baseHi, baseB=baseB,
                offsLo=offsLo, offsHi=offsHi, offsB=offsB,
                idxLo=idxLo, idxHi=idxHi, idxB=idxB,
                recipA=recipA, recipB=recipB, nord=nord)


def _emulate(pp, inp_f32, weight, bias):
    """Numpy emulation of the exact device program (for logic validation)."""
    K_lo, K_hi, K_B = pp['K_lo'], pp['K_hi'], pp['K_B']
    baseLo, baseHi, baseB = pp['baseLo'], pp['baseHi'], pp['baseB']
    lo = np.concatenate([inp_f32[:SPLIT], np.zeros((1, D), np.float32)], 0)
    hi = np.concatenate([inp_f32[SPLIT:], np.zeros((1, D), np.float32)], 0)
    y_full = np.zeros((RANKS_E, D), np.float32)
    for c in range(NCORES):
        for w in range(W_A):
            s = np.zeros((P, D), np.float32)
            if K_lo[w]:
                s += lo[pp['offsLo'][c][:, baseLo[w]:baseLo[w + 1]]].sum(1)
            if K_hi[w]:
                s += hi[pp['offsHi'][c][:, baseHi[w]:baseHi[w + 1]]].sum(1)
            y = s * pp['recipA'][c][:, w][:, None]
            yw = y @ weight
            for p in range(P):
                rank = w * NCORES * P + c * P + p
                y_full[_y_row(rank)] = yw[p]
    out = np.zeros((N_NODE, D), np.float32)
    for c in range(NCORES):
        for w in range(W_B):
            g = y_full[pp['offsB'][c][:, baseB[w]:baseB[w + 1]]]
            s = g.sum(1)
            z = s * pp['recipB'][c][:, w][:, None] + bias[None, :]
            for p in range(P):
                rank = w * NCORES * P + c * P + p
                if rank < N_NODE:
                    out[pp['nord'][rank]] = z[p]
    return out


def _build_program(K_lo, K_hi, K_B, rep=1, no_coll=False):
    import concourse.bacc as bacc
    import concourse.bass as bass
    import concourse.tile as tile
    from concourse import mybir

    f32 = mybir.dt.float32
    bf16 = mybir.dt.bfloat16
    i16 = mybir.dt.int16
    add = mybir.AluOpType.add
    Copy = mybir.ActivationFunctionType.Copy

    CLo = 8 * sum(K_lo)
    CHi = 8 * sum(K_hi)
    CB = 8 * sum(K_B)
    bLo = np.concatenate([[0], np.cumsum(K_lo)]).astype(int)
    bHi = np.concatenate([[0], np.cumsum(K_hi)]).astype(int)
    bB = np.concatenate([[0], np.cumsum(K_B)]).astype(int)

    nc = bacc.Bacc(None, target_bir_lowering=False, debug=False)
    inp_lo = nc.dram_tensor("inp_lo", [LO_ROWS, D], bf16,
                            kind="ExternalInput")
    inp_hi = nc.dram_tensor("inp_hi", [HI_ROWS, D], bf16,
                            kind="ExternalInput")
    wgt = nc.dram_tensor("wgt", [D, D], bf16, kind="ExternalInput")
    bias_bc = nc.dram_tensor("bias_bc", [P, D], f32, kind="ExternalInput")
    ident_in = nc.dram_tensor("ident_in", [P, P], bf16, kind="ExternalInput")
    idxLo_d = nc.dram_tensor("idxLo", [P, CLo], i16, kind="ExternalInput")
    idxHi_d = nc.dram_tensor("idxHi", [P, CHi], i16, kind="ExternalInput")
    idxB_d = nc.dram_tensor("idxB", [P, CB], i16, kind="ExternalInput")
    recipA_d = nc.dram_tensor("recipA", [P, W_A], f32, kind="ExternalInput")
    recipB_d = nc.dram_tensor("recipB", [P, W_B], f32, kind="ExternalInput")
    out = nc.dram_tensor("out", [NSH, D], f32, kind="ExternalOutput")

    with tile.TileContext(nc) as tc:
        with tc.tile_pool(name="const", bufs=1) as cpool, \
             tc.tile_pool(name="gat", bufs=3) as gpool, \
             tc.tile_pool(name="sred", bufs=3) as spool, \
             tc.tile_pool(name="yst", bufs=4) as ypool, \
             tc.tile_pool(name="res", bufs=4) as respool, \
             tc.tile_pool(name="ps", bufs=4, space="PSUM") as ppool, \
             tc.tile_pool(name="dram", bufs=1, space="DRAM") as dpool:

            wgt_t = cpool.tile([D, D], bf16)
            nc.sync.dma_start(out=wgt_t[:], in_=wgt[:])
            bias_t = cpool.tile([P, D], f32)
            nc.sync.dma_start(out=bias_t[:], in_=bias_bc[:])
            ident_t = cpool.tile([P, P], bf16)
            nc.sync.dma_start(out=ident_t[:], in_=ident_in[:])
            idxLo_t = cpool.tile([P, CLo], i16)
            nc.sync.dma_start(out=idxLo_t[:], in_=idxLo_d[:])
            idxHi_t = cpool.tile([P, CHi], i16)
            nc.sync.dma_start(out=idxHi_t[:], in_=idxHi_d[:])
            idxB_t = cpool.tile([P, CB], i16)
            nc.sync.dma_start(out=idxB_t[:], in_=idxB_d[:])
            recipA_t = cpool.tile([P, W_A], f32)
            nc.sync.dma_start(out=recipA_t[:], in_=recipA_d[:])
            recipB_t = cpool.tile([P, W_B], f32)
            nc.sync.dma_start(out=recipB_t[:], in_=recipB_d[:])

            y_shard = dpool.tile([ESH, D], bf16)
            y_full = dpool.tile([RANKS_E, D], bf16, name="y_full")
            y_ch_all = [[dpool.tile([NCORES * CH_E, D], bf16,
                                    addr_space="Shared",
                                    name=f"y_ch{k}_{r}") for k in range(NCH)]
                        for r in range(rep)]

            KC = 8    # dma_gather num_idxs limit: 1024 = KC * 128

            def gather(g, col0, K, src, idx_t, ib, tag):
                for k0 in range(0, K, KC):
                    kc = min(KC, K - k0)
                    nc.gpsimd.dma_gather(
                        out_ap=g[:, (col0 + k0) * D:
                                 (col0 + k0 + kc) * D].rearrange(
                            "p (k d) -> p k d", d=D),
                        in_ap=src[:],
                        idxs_ap=idx_t[:, 8 * (ib + k0):8 * (ib + k0 + kc)],
                        num_idxs=kc * P, num_idxs_reg=kc * P, elem_size=D)

            def tree_reduce(g, K, tag):
                blocks = K
                while blocks > 2:
                    h = blocks // 2
                    nc.vector.tensor_tensor(
                        out=g[:, :h * D], in0=g[:, :h * D],
                        in1=g[:, (blocks - h) * D:blocks * D], op=add)
                    blocks -= h
                if blocks == 2:
                    s = spool.tile([P, D], f32, name=f"s_{tag}", tag=f"s_{tag}")
                    nc.vector.tensor_tensor(out=s[:], in0=g[:, :D],
                                            in1=g[:, D:2 * D], op=add)
                    return s[:]
                return g[:, :D]

            for _r in range(rep):
                y_ch = y_ch_all[_r]
                # ---------------- Phase A ----------------
                for w in range(W_A):
                    KL, KH = K_lo[w], K_hi[w]
                    KT = KL + KH
                    g = gpool.tile([P, KT * D], bf16, name="gA", tag="gA")
                    if KL:
                        gather(g, 0, KL, inp_lo, idxLo_t, bLo[w], "lo")
                    if KH:
                        gather(g, KL, KH, inp_hi, idxHi_t, bHi[w], "hi")
                    src = tree_reduce(g, KT, "a")
                    y = ypool.tile([P, D], bf16, name="yA", tag="yA")
                    nc.scalar.activation(out=y[:], in_=src, func=Copy,
                                         scale=recipA_t[:, w:w + 1])
                    yT_p = ppool.tile([P, D], bf16, name="yTp", tag="yTp")
                    nc.tensor.transpose(out=yT_p[:], in_=y[:],
                                        identity=ident_t[:])
                    yT = ypool.tile([P, D], bf16, name="yT", tag="yT")
                    nc.scalar.activation(out=yT[:], in_=yT_p[:], func=Copy)
                    yw_p = ppool.tile([P, D], f32, name="ywp", tag="ywp")
                    nc.tensor.matmul(yw_p[:], lhsT=yT[:], rhs=wgt_t[:],
                                     start=True, stop=True)
                    yw = ypool.tile([P, D], bf16, name="ywA", tag="ywA")
                    nc.scalar.activation(out=yw[:], in_=yw_p[:], func=Copy)
                    nc.sync.dma_start(out=y_shard[w * P:(w + 1) * P, :],
                                      in_=yw[:])
                    if (w + 1) % (W_A // NCH) == 0:
                        k = w // (W_A // NCH)
                        if no_coll:
                            nc.sync.dma_start(
                                out=y_full[k * NCORES * CH_E:
                                           k * NCORES * CH_E + CH_E, :],
                                in_=y_shard[k * CH_E:(k + 1) * CH_E, :])
                        else:
                            nc.gpsimd.collective_compute(
                                "AllGather", mybir.AluOpType.bypass,
                                replica_groups=[list(range(NCORES))],
                                ins=[y_shard[k * CH_E:(k + 1) * CH_E, :]],
                                outs=[y_ch[k].opt()])
                            nc.sync.dma_start(
                                out=y_full[k * NCORES * CH_E:
                                           (k + 1) * NCORES * CH_E, :],
                                in_=y_ch[k][:])

                # ---------------- Phase B ----------------
                for w in range(W_B):
                    KB = K_B[w]
                    g2 = gpool.tile([P, KB * D], bf16, name="gB", tag="gB")
                    gather(g2, 0, KB, y_full, idxB_t, bB[w], "b")
                    src2 = tree_reduce(g2, KB, "b")
                    z = respool.tile([P, D], f32, name="zB", tag="zB")
                    nc.scalar.activation(out=z[:], in_=src2, func=Copy,
                                         scale=recipB_t[:, w:w + 1])
                    res = respool.tile([P, D], f32, name="resB", tag="resB")
                    nc.vector.tensor_tensor(out=res[:], in0=z[:],
                                            in1=bias_t[:], op=add)
                    nc.sync.dma_start(out=out[w * P:(w + 1) * P, :],
                                      in_=res[:])

    nc.compile()
    return nc


def kernel(input, weight, bias, V, E, num_edges):
    global LAST_RESULTS
    inp = np.ascontiguousarray(np.asarray(input), dtype=np.float32)
    wgt = np.ascontiguousarray(np.asarray(weight), dtype=np.float32)
    b = np.asarray(bias).astype(np.float32)
    pp = _preprocess(V, E)

    if os.environ.get('KERNEL_EMULATE'):
        return _emulate(pp, inp, wgt, b)

    from concourse.bass_utils import run_bass_kernel_spmd

    key = (tuple(pp['K_lo']), tuple(pp['K_hi']), tuple(pp['K_B']))
    if key not in _PROG_CACHE:
        _PROG_CACHE[key] = _build_program(*key)
    nc = _PROG_CACHE[key]

    import ml_dtypes
    bf = ml_dtypes.bfloat16
    lo = np.concatenate([inp[:SPLIT], np.zeros((1, D), np.float32)],
                        0).astype(bf)
    hi = np.concatenate([inp[SPLIT:], np.zeros((1, D), np.float32)],
                        0).astype(bf)
    bias_bc = np.tile(b[None, :], (P, 1)).astype(np.float32)
    ident = np.eye(P, dtype=np.float32).astype(bf)
    in_maps = []
    for c in range(NCORES):
        in_maps.append(dict(
            inp_lo=lo, inp_hi=hi, wgt=wgt.astype(bf), bias_bc=bias_bc,
            ident_in=ident,
            idxLo=pp['idxLo'][c], idxHi=pp['idxHi'][c], idxB=pp['idxB'][c],
            recipA=pp['recipA'][c], recipB=pp['recipB'][c]))

    trace = bool(os.environ.get('KERNEL_TRACE'))
    res = run_bass_kernel_spmd(nc, in_maps, list(range(NCORES)), trace=trace)
    LAST_RESULTS = res

    out_full = np.zeros((N_NODE, D), np.float32)
    nord = pp['nord']
    for c in range(NCORES):
        rows = np.asarray(res.results[c]['out'])          # [NSH, D]
        ranks = (np.arange(NSH) // P) * (NCORES * P) + c * P + np.arange(NSH) % P
        m = ranks < N_NODE
        out_full[nord[ranks[m]]] = rows[m]
    return out_full


# revision 19
# speedup vs baseline: 4.9525x; 4.0312x over previous
"""HGNN conv on 8 trn2 cores — v3 (dma_gather batched window gathers).

out = D_v^-1 H D_e^-1 H^T input W + bias   (W applied to edge features y)

Phase A (edge-sharded): edges sorted by (lo,hi) incidence counts (desc,
where lo counts entries with V < 32767 and hi the rest — dma_gather
indices are int16 so the 50001-row input is split into inp_lo/inp_hi,
each with a trailing zero row for padding) and dealt round-robin to 8
cores in 128-edge windows. Per window: one dma_gather from inp_lo
(K_lo*128 rows) + one from inp_hi (K_hi*128 rows) into a [P, K*D] tile;
pairwise bf16 add-tree reduces over K; recip-scale on the Act engine; W
applied via PE transpose + matmul; rows staged to y_shard. Every 5
windows a chunk AllGathers into Shared y_ch -> copied into y_full.
Phase B (node-sharded): nodes bucketed by count; one dma_gather per
window from y_full (25600 rows, int16-safe), tree-reduce, recip-scale,
add bias, store. Output rows are host-side inverse-permuted.
"""
import os
import sys

for _p in ('/opt/trn_rl_repo', '/root/.axon_site/_ro/trn_rl_repo'):
    if os.path.isdir(_p) and _p not in sys.path:
        sys.path.insert(0, _p)

import numpy as np

P = 128
NCORES = 8
N_NODE = 50000
N_EDGE = 25000
D = 128
SPLIT = 32767         # input rows [0, SPLIT) -> inp_lo, [SPLIT, N) -> inp_hi
LO_ROWS = SPLIT + 1   # + zero row at index SPLIT
HI_ROWS = N_NODE - SPLIT + 1  # + zero row at index N_NODE - SPLIT
W_A = 25              # edge windows per core (8*25*128 = 25600 slots)
RANKS_E = NCORES * W_A * P
ESH = W_A * P         # 3200 edge slots per core
NCH = 5               # allgather chunks (5 windows each)
CH_E = ESH // NCH     # 640 edge rows per chunk per core
W_B = 49              # node windows per core (8*49*128 = 50176 slots)
RANKS_V = NCORES * W_B * P
NSH = W_B * P         # 6272 node slots per core

_PROG_CACHE = {}
LAST_RESULTS = None


def _y_row(rank):
    """y_full row for global edge rank, chunk-major allgather layout."""
    c = (rank // P) % NCORES
    w = rank // (NCORES * P)
    p = rank % P
    k = w // (W_A // NCH)
    return (k * (NCORES * CH_E) + c * CH_E + (w % (W_A // NCH)) * P + p)


def _pack_idx(mat):
    """[P, K] int -> [128, 8*K] int16: flat index i=k*128+p lives at
    [i%16, i//16], replicated across the 8 16-partition groups."""
    Pn, K = mat.shape
    assert Pn == P
    flat = mat.T.reshape(-1)             # i = k*128 + p
    arr = flat.reshape(-1, 16).T         # [16, 8*K]
    return np.ascontiguousarray(np.tile(arr, (8, 1)).astype(np.int16))


def _fill_cols(dst, base, K, rs, j, vals, blk):
    """Scatter vals into dst[c][p, 8-packed cols] — done at int-matrix level
    by the caller; helper kept trivial."""
    raise NotImplementedError


def _preprocess(V, E):
    V = np.asarray(V).astype(np.int64)
    E = np.asarray(E).astype(np.int64)
    nnz = len(V)

    cntV = np.bincount(V, minlength=N_NODE)
    is_lo = V < SPLIT
    lo_cnt = np.bincount(E[is_lo], minlength=N_EDGE)
    hi_cnt = np.bincount(E[~is_lo], minlength=N_EDGE)

    # ----- Phase A: bucket edges by (lo, hi) counts desc -----
    eord = np.lexsort((-hi_cnt, -lo_cnt))            # rank -> edge id
    rankE = np.empty(N_EDGE, np.int64)
    rankE[eord] = np.arange(N_EDGE)
    lo_r = np.zeros(RANKS_E, np.int64)
    lo_r[:N_EDGE] = lo_cnt[eord]
    hi_r = np.zeros(RANKS_E, np.int64)
    hi_r[:N_EDGE] = hi_cnt[eord]
    blk = NCORES * P                                 # 1024 ranks per window row
    K_lo, K_hi = [], []
    for w in range(W_A):
        kl = int(lo_r[w * blk:(w + 1) * blk].max())
        kh = int(hi_r[w * blk:(w + 1) * blk].max())
        if kl + kh == 0:
            kh = 1
        K_lo.append(kl)
        K_hi.append(kh)
    baseLo = np.concatenate([[0], np.cumsum(K_lo)]).astype(np.int64)
    baseHi = np.concatenate([[0], np.cumsum(K_hi)]).astype(np.int64)
    SLo, SHi = int(baseLo[-1]), int(baseHi[-1])

    offsLo = np.full((NCORES, P, SLo), SPLIT, np.int64)
    offsHi = np.full((NCORES, P, SHi), N_NODE - SPLIT, np.int64)

    def scatter(mask, offs, base, vals):
        ordx = np.argsort(rankE[E[mask]], kind='stable')
        rsx = rankE[E[mask]][ordx]
        startsx = np.searchsorted(rsx, np.arange(RANKS_E))
        jx = np.arange(len(rsx)) - startsx[rsx]
        cx = (rsx // P) % NCORES
        wx = rsx // blk
        px = rsx % P
        offs[cx, px, base[wx] + jx] = vals[mask][ordx]

    scatter(is_lo, offsLo, baseLo, V)
    scatter(~is_lo, offsHi, baseHi, V - SPLIT)

    cnt_r = lo_r + hi_r
    recipA_flat = np.where(cnt_r > 0, 1.0 / np.maximum(cnt_r, 1), 0.0)
    recipA = recipA_flat.reshape(W_A, NCORES, P).transpose(1, 2, 0)
    recipA = np.ascontiguousarray(recipA, np.float32)

    # ----- Phase B: bucket nodes by count desc -----
    nord = np.argsort(-cntV, kind='stable')
    rankV = np.empty(N_NODE, np.int64)
    rankV[nord] = np.arange(N_NODE)
    cnt2_r = np.zeros(RANKS_V, np.int64)
    cnt2_r[:N_NODE] = cntV[nord]
    K_B = [max(1, int(cnt2_r[w * blk])) for w in range(W_B)]
    baseB = np.concatenate([[0], np.cumsum(K_B)]).astype(np.int64)
    SB = int(baseB[-1])

    pad_row = _y_row(RANKS_E - 1)                    # a guaranteed-zero y row
    offsB = np.full((NCORES, P, SB), pad_row, np.int64)
    ordB = np.argsort(rankV[V], kind='stable')
    rs2 = rankV[V[ordB]]
    starts2 = np.searchsorted(rs2, np.arange(RANKS_V))
    j2 = np.arange(nnz) - starts2[rs2]
    c2 = (rs2 // P) % NCORES
    w2 = rs2 // blk
    p2 = rs2 % P
    offsB[c2, p2, baseB[w2] + j2] = _y_row(rankE[E[ordB]])

    recipB_flat = np.where(cnt2_r > 0, 1.0 / np.maximum(cnt2_r, 1), 0.0)
    recipB = recipB_flat.reshape(W_B, NCORES, P).transpose(1, 2, 0)
    recipB = np.ascontiguousarray(recipB, np.float32)

    # pack per-core int16 index tiles
    idxLo = np.stack([np.concatenate(
        [_pack_idx(offsLo[c][:, baseLo[w]:baseLo[w + 1]]) for w in range(W_A)
         if K_lo[w] > 0] or [np.zeros((P, 0), np.int16)], 1)
        for c in range(NCORES)])
    idxHi = np.stack([np.concatenate(
        [_pack_idx(offsHi[c][:, baseHi[w]:baseHi[w + 1]]) for w in range(W_A)
         if K_hi[w] > 0] or [np.zeros((P, 0), np.int16)], 1)
        for c in range(NCORES)])
    idxB = np.stack([np.concatenate(
        [_pack_idx(offsB[c][:, baseB[w]:baseB[w + 1]]) for w in range(W_B)], 1)
        for c in range(NCORES)])

    return dict(K_lo=K_lo, K_hi=K_hi, K_B=K_B,
                baseLo=baseLo, baseHi=baseHi, baseB=baseB,
                offsLo=offsLo, offsHi=offsHi, offsB=offsB,
                idxLo=idxLo, idxHi=idxHi, idxB=idxB,
                recipA=recipA, recipB=recipB, nord=nord)


def _emulate(pp, inp_f32, weight, bias):
    """Numpy emulation of the exact device program (for logic validation)."""
    K_lo, K_hi, K_B = pp['K_lo'], pp['K_hi'], pp['K_B']
    baseLo, baseHi, baseB = pp['baseLo'], pp['baseHi'], pp['baseB']
    lo = np.concatenate([inp_f32[:SPLIT], np.zeros((1, D), np.float32)], 0)
    hi = np.concatenate([inp_f32[SPLIT:], np.zeros((1, D), np.float32)], 0)
    y_full = np.zeros((RANKS_E, D), np.float32)
    for c in range(NCORES):
        for w in range(W_A):
            s = np.zeros((P, D), np.float32)
            if K_lo[w]:
                s += lo[pp['offsLo'][c][:, baseLo[w]:baseLo[w + 1]]].sum(1)
            if K_hi[w]:
                s += hi[pp['offsHi'][c][:, baseHi[w]:baseHi[w + 1]]].sum(1)
            y = s * pp['recipA'][c][:, w][:, None]
            yw = y @ weight
            for p in range(P):
                rank = w * NCORES * P + c * P + p
                y_full[_y_row(rank)] = yw[p]
    out = np.zeros((N_NODE, D), np.float32)
    for c in range(NCORES):
        for w in range(W_B):
            g = y_full[pp['offsB'][c][:, baseB[w]:baseB[w + 1]]]
            s = g.sum(1)
            z = s * pp['recipB'][c][:, w][:, None] + bias[None, :]
            for p in range(P):
                rank = w * NCORES * P + c * P + p
                if rank < N_NODE:
                    out[pp['nord'][rank]] = z[p]
    return out


def _build_program(K_lo, K_hi, K_B, rep=1, no_coll=False,
                   skip_gather=False, skip_compute=False):
    import concourse.bacc as bacc
    import concourse.bass as bass
    import concourse.tile as tile
    from concourse import mybir

    f32 = mybir.dt.float32
    bf16 = mybir.dt.bfloat16
    i16 = mybir.dt.int16
    add = mybir.AluOpType.add
    Copy = mybir.ActivationFunctionType.Copy

    CLo = 8 * sum(K_lo)
    CHi = 8 * sum(K_hi)
    CB = 8 * sum(K_B)
    bLo = np.concatenate([[0], np.cumsum(K_lo)]).astype(int)
    bHi = np.concatenate([[0], np.cumsum(K_hi)]).astype(int)
    bB = np.concatenate([[0], np.cumsum(K_B)]).astype(int)

    NQ = 4
    nc = bacc.Bacc(None, target_bir_lowering=False, debug=False,
                   num_swdge_queues=NQ)
    nc.m.attributes = (nc.m.attributes or {}) | {"num_swdge_queues": NQ}
    inp_lo = nc.dram_tensor("inp_lo", [LO_ROWS, D], bf16,
                            kind="ExternalInput")
    inp_hi = nc.dram_tensor("inp_hi", [HI_ROWS, D], bf16,
                            kind="ExternalInput")
    wgt = nc.dram_tensor("wgt", [D, D], bf16, kind="ExternalInput")
    bias_bc = nc.dram_tensor("bias_bc", [P, D], f32, kind="ExternalInput")
    ident_in = nc.dram_tensor("ident_in", [P, P], bf16, kind="ExternalInput")
    idxLo_d = nc.dram_tensor("idxLo", [P, CLo], i16, kind="ExternalInput")
    idxHi_d = nc.dram_tensor("idxHi", [P, CHi], i16, kind="ExternalInput")
    idxB_d = nc.dram_tensor("idxB", [P, CB], i16, kind="ExternalInput")
    recipA_d = nc.dram_tensor("recipA", [P, W_A], f32, kind="ExternalInput")
    recipB_d = nc.dram_tensor("recipB", [P, W_B], f32, kind="ExternalInput")
    out = nc.dram_tensor("out", [NSH, D], f32, kind="ExternalOutput")

    with tile.TileContext(nc) as tc:
        with tc.tile_pool(name="const", bufs=1) as cpool, \
             tc.tile_pool(name="gat", bufs=3) as gpool, \
             tc.tile_pool(name="sred", bufs=3) as spool, \
             tc.tile_pool(name="yst", bufs=4) as ypool, \
             tc.tile_pool(name="res", bufs=4) as respool, \
             tc.tile_pool(name="ps", bufs=4, space="PSUM") as ppool, \
             tc.tile_pool(name="dram", bufs=1, space="DRAM") as dpool:

            wgt_t = cpool.tile([D, D], bf16)
            nc.sync.dma_start(out=wgt_t[:], in_=wgt[:])
            bias_t = cpool.tile([P, D], f32)
            nc.sync.dma_start(out=bias_t[:], in_=bias_bc[:])
            ident_t = cpool.tile([P, P], bf16)
            nc.sync.dma_start(out=ident_t[:], in_=ident_in[:])
            idxLo_t = cpool.tile([P, CLo], i16)
            nc.sync.dma_start(out=idxLo_t[:], in_=idxLo_d[:])
            idxHi_t = cpool.tile([P, CHi], i16)
            nc.sync.dma_start(out=idxHi_t[:], in_=idxHi_d[:])
            idxB_t = cpool.tile([P, CB], i16)
            nc.sync.dma_start(out=idxB_t[:], in_=idxB_d[:])
            recipA_t = cpool.tile([P, W_A], f32)
            nc.sync.dma_start(out=recipA_t[:], in_=recipA_d[:])
            recipB_t = cpool.tile([P, W_B], f32)
            nc.sync.dma_start(out=recipB_t[:], in_=recipB_d[:])

            y_shard = dpool.tile([ESH, D], bf16)
            y_full = dpool.tile([RANKS_E, D], bf16, name="y_full")
            y_ch_all = [[dpool.tile([NCORES * CH_E, D], bf16,
                                    addr_space="Shared",
                                    name=f"y_ch{k}_{r}") for k in range(NCH)]
                        for r in range(rep)]

            KC = 8    # dma_gather num_idxs limit: 1024 = KC * 128
            qctr = [0]

            def gather(g, col0, K, src, idx_t, ib, tag):
                if skip_gather:
                    return
                for k0 in range(0, K, KC):
                    kc = min(KC, K - k0)
                    nc.gpsimd.dma_gather(
                        out_ap=g[:, (col0 + k0) * D:
                                 (col0 + k0 + kc) * D].rearrange(
                            "p (k d) -> p k d", d=D),
                        in_ap=src[:],
                        idxs_ap=idx_t[:, 8 * (ib + k0):8 * (ib + k0 + kc)],
                        num_idxs=kc * P, num_idxs_reg=kc * P, elem_size=D,
                        queue_num=qctr[0] % NQ)
                    qctr[0] += 1

            def tree_reduce(g, K, tag):
                blocks = K
                while blocks > 2:
                    h = blocks // 2
                    nc.vector.tensor_tensor(
                        out=g[:, :h * D], in0=g[:, :h * D],
                        in1=g[:, (blocks - h) * D:blocks * D], op=add)
                    blocks -= h
                if blocks == 2:
                    s = spool.tile([P, D], f32, name=f"s_{tag}", tag=f"s_{tag}")
                    nc.vector.tensor_tensor(out=s[:], in0=g[:, :D],
                                            in1=g[:, D:2 * D], op=add)
                    return s[:]
                return g[:, :D]

            for _r in range(rep):
                y_ch = y_ch_all[_r]
                # ---------------- Phase A ----------------
                for w in range(W_A):
                    KL, KH = K_lo[w], K_hi[w]
                    KT = KL + KH
                    g = gpool.tile([P, KT * D], bf16, name="gA", tag="gA")
                    if KL:
                        gather(g, 0, KL, inp_lo, idxLo_t, bLo[w], "lo")
                    if KH:
                        gather(g, KL, KH, inp_hi, idxHi_t, bHi[w], "hi")
                    if skip_compute:
                        yw = ypool.tile([P, D], bf16, name="ywA", tag="ywA")
                        nc.vector.tensor_copy(out=yw[:], in_=g[:, :D])
                    else:
                        src = tree_reduce(g, KT, "a")
                        y = ypool.tile([P, D], bf16, name="yA", tag="yA")
                        nc.scalar.activation(out=y[:], in_=src, func=Copy,
                                             scale=recipA_t[:, w:w + 1])
                        yT_p = ppool.tile([P, D], bf16, name="yTp", tag="yTp")
                        nc.tensor.transpose(out=yT_p[:], in_=y[:],
                                            identity=ident_t[:])
                        yT = ypool.tile([P, D], bf16, name="yT", tag="yT")
                        nc.scalar.activation(out=yT[:], in_=yT_p[:], func=Copy)
                        yw_p = ppool.tile([P, D], f32, name="ywp", tag="ywp")
                        nc.tensor.matmul(yw_p[:], lhsT=yT[:], rhs=wgt_t[:],
                                         start=True, stop=True)
                        yw = ypool.tile([P, D], bf16, name="ywA", tag="ywA")
                        nc.scalar.activation(out=yw[:], in_=yw_p[:], func=Copy)
                    nc.sync.dma_start(out=y_shard[w * P:(w + 1) * P, :],
                                      in_=yw[:])
                    if (w + 1) % (W_A // NCH) == 0:
                        k = w // (W_A // NCH)
                        if no_coll:
                            nc.sync.dma_start(
                                out=y_full[k * NCORES * CH_E:
                                           k * NCORES * CH_E + CH_E, :],
                                in_=y_shard[k * CH_E:(k + 1) * CH_E, :])
                        else:
                            nc.gpsimd.collective_compute(
                                "AllGather", mybir.AluOpType.bypass,
                                replica_groups=[list(range(NCORES))],
                                ins=[y_shard[k * CH_E:(k + 1) * CH_E, :]],
                                outs=[y_ch[k].opt()])
                            nc.sync.dma_start(
                                out=y_full[k * NCORES * CH_E:
                                           (k + 1) * NCORES * CH_E, :],
                                in_=y_ch[k][:])

                # ---------------- Phase B ----------------
                for w in range(W_B):
                    KB = K_B[w]
                    g2 = gpool.tile([P, KB * D], bf16, name="gB", tag="gB")
                    gather(g2, 0, KB, y_full, idxB_t, bB[w], "b")
                    if skip_compute:
                        res = respool.tile([P, D], f32, name="resB",
                                           tag="resB")
                        nc.vector.tensor_copy(out=res[:], in_=g2[:, :D])
                    else:
                        src2 = tree_reduce(g2, KB, "b")
                        z = respool.tile([P, D], f32, name="zB", tag="zB")
                        nc.scalar.activation(out=z[:], in_=src2, func=Copy,
                                             scale=recipB_t[:, w:w + 1])
                        res = respool.tile([P, D], f32, name="resB",
                                           tag="resB")
                        nc.vector.tensor_tensor(out=res[:], in0=z[:],
                                                in1=bias_t[:], op=add)
                    nc.sync.dma_start(out=out[w * P:(w + 1) * P, :],
                                      in_=res[:])

    nc.compile()
    return nc


def kernel(input, weight, bias, V, E, num_edges):
    global LAST_RESULTS
    inp = np.ascontiguousarray(np.asarray(input), dtype=np.float32)
    wgt = np.ascontiguousarray(np.asarray(weight), dtype=np.float32)
    b = np.asarray(bias).astype(np.float32)
    pp = _preprocess(V, E)

    if os.environ.get('KERNEL_EMULATE'):
        return _emulate(pp, inp, wgt, b)

    from concourse.bass_utils import run_bass_kernel_spmd

    key = (tuple(pp['K_lo']), tuple(pp['K_hi']), tuple(pp['K_B']))
    if key not in _PROG_CACHE:
        _PROG_CACHE[key] = _build_program(*key)
    nc = _PROG_CACHE[key]

    import ml_dtypes
    bf = ml_dtypes.bfloat16
    lo = np.concatenate([inp[:SPLIT], np.zeros((1, D), np.float32)],
                        0).astype(bf)
    hi = np.concatenate([inp[SPLIT:], np.zeros((1, D), np.float32)],
                        0).astype(bf)
    bias_bc = np.tile(b[None, :], (P, 1)).astype(np.float32)
    ident = np.eye(P, dtype=np.float32).astype(bf)
    in_maps = []
    for c in range(NCORES):
        in_maps.append(dict(
            inp_lo=lo, inp_hi=hi, wgt=wgt.astype(bf), bias_bc=bias_bc,
            ident_in=ident,
            idxLo=pp['idxLo'][c], idxHi=pp['idxHi'][c], idxB=pp['idxB'][c],
            recipA=pp['recipA'][c], recipB=pp['recipB'][c]))

    trace = bool(os.environ.get('KERNEL_TRACE'))
    res = run_bass_kernel_spmd(nc, in_maps, list(range(NCORES)), trace=trace)
    LAST_RESULTS = res

    out_full = np.zeros((N_NODE, D), np.float32)
    nord = pp['nord']
    for c in range(NCORES):
        rows = np.asarray(res.results[c]['out'])          # [NSH, D]
        ranks = (np.arange(NSH) // P) * (NCORES * P) + c * P + np.arange(NSH) % P
        m = ranks < N_NODE
        out_full[nord[ranks[m]]] = rows[m]
    return out_full
